# revision 31
# baseline (speedup 1.0000x reference)
"""AnchorTargetLayer Trainium2 kernel.

Data-parallel over batch: 32 images / 8 NeuronCores = 4 images per core.
All 5 anchor levels are concatenated into one 65472-anchor axis (padded to
65536) — the per-level structure only matters for the forced-positive
(per-GT best anchor) rule, and argmax-over-levels of per-level maxima is
identical to a single global argmax over the concatenated axis.

Per 128x16-anchor tile, all 4 images are processed in one instruction
(free size 4*16*20). Ordering/thresholds use r = inter/(area_a+area_g),
a strictly monotone transform of IoU (r = ov/(1+ov)), so no division by
the union is needed; thresholds 0.4/0.5 become 2/7 and 1/3.

Engine split: GpSimd takes the min/add front half of the IoU, ScalarE the
relus, DVE the multiplies/reduces/compares and the one-hot gather of
(gcx, gcy, log gw, log gh, id) for assigned GTs.

The per-GT argmax over anchors is resolved on host: the device emits the
per-(tile, partition) max of r for each (image, gt); the host picks the
winning row (first-index ties, matching the reference) and recomputes the
16 IoUs of that row exactly to find the winning anchor, then applies the
20-per-image forced-positive patches.
"""

import sys

import numpy as np

sys.path.insert(0, "/opt/trn_rl_repo")

import concourse.bass as bass
import concourse.mybir as mybir
from concourse.bass_utils import run_bass_kernel_spmd
from concourse.tile import TileContext

F32 = mybir.dt.float32
I32 = mybir.dt.int32

N_CORES = 8
B = 32
G = 20
NB = B // N_CORES          # images per core
A_TOT = 65472              # 49152 + 12288 + 3072 + 768 + 192
A_PAD = 65536
T = 32                     # anchors per partition per tile
NT = A_PAD // (128 * T)    # 16 tiles
NCOL = 13                  # anchor-ext columns
POS_R = float(np.float32(1.0 / 3.0))   # ov >= 0.5  <=>  r >= 1/3
IGN_R = float(np.float32(2.0 / 7.0))   # ov >= 0.4  <=>  r >= 2/7

_BUILD_CACHE = {}


def _build_kernel():
    if "nc" in _BUILD_CACHE:
        return _BUILD_CACHE["nc"]
    nc = bass.Bass()
    an = nc.declare_dram_parameter("anch", [A_PAD, NCOL], F32, isOutput=False)
    gt = nc.declare_dram_parameter("gte", [NB, 128, NCOL * G], F32, isOutput=False)
    bd = nc.declare_dram_parameter("bdg", [128, NB * G], F32, isOutput=False)
    idn = nc.declare_dram_parameter("iden", [128, 128], F32, isOutput=False)
    sel = nc.declare_dram_parameter("sel", [2 * T, NB * T * G], F32, isOutput=False)
    lab_o = nc.declare_dram_parameter("labels", [NB, A_PAD], I32, isOutput=True)
    reg_o = nc.declare_dram_parameter("reg", [NB, A_PAD, 4], F32, isOutput=True)
    aux_o = nc.declare_dram_parameter("aux", [NT, 128, NB * G], F32, isOutput=True)

    tt = mybir.AluOpType
    FS = [128, NB, T, G]

    with TileContext(nc) as tc:
        with (
            tc.tile_pool(name="res", bufs=1) as rp,
            tc.tile_pool(name="wk", bufs=2) as wp,
            tc.tile_pool(name="sm", bufs=3) as sp,
            tc.tile_pool(name="ot", bufs=3) as op,
            tc.tile_pool(name="ps", bufs=2, space="PSUM") as pp,
        ):
            anch = rp.tile([128, NT, T, NCOL], F32, tag="anch")
            for n in range(NT):
                nc.sync.dma_start(
                    out=anch[:, n],
                    in_=an[n * 128 * T:(n + 1) * 128 * T].rearrange(
                        "(p t) c -> p t c", p=128),
                )
            gtb = rp.tile([128, NB, NCOL, G], F32, tag="gtb")
            for b in range(NB):
                nc.sync.dma_start(
                    out=gtb[:, b],
                    in_=gt[b].rearrange("p (c g) -> p c g", g=G),
                )
            bdt = rp.tile([128, NB, G], F32, tag="bdt")
            nc.sync.dma_start(out=bdt[:], in_=bd[:].rearrange("p (b g) -> p b g", g=G))
            iden = rp.tile([128, 128], F32, tag="iden")
            nc.sync.dma_start(out=iden[:], in_=idn[:])
            selt = rp.tile([2 * T, NB * T * G], F32, tag="selt")
            nc.sync.dma_start(out=selt[:], in_=sel[:])
            eq32s = [rp.tile([128, NB, T, 32], F32, name=f"eq32_{i}", tag=f"eq32_{i}")
                     for i in range(2)]
            for e in eq32s:
                nc.vector.memset(e[:], 0.0)

            def phase_rcp(n):
                at = anch[:, n]
                ltin = wp.tile([128, 2, T], F32, name="ltin", tag="ltin")
                nc.scalar.copy(ltin[:], at[:, :, 4:13:7].rearrange("p t c -> p c t"))
                lt_ps = pp.tile([2 * T, 128], F32, name="lt_ps", tag="lt_ps", bufs=1)
                nc.tensor.transpose(
                    lt_ps[:], ltin[:].rearrange("p c t -> p (c t)"), iden[:])
                lt = wp.tile([2 * T, 128], F32, name="lt", tag="lt")
                nc.scalar.copy(lt[:], lt_ps[:])
                rcp = wp.tile(FS, F32, name="rcp", tag="rcp")
                for half in range(2):
                    sm_ps = pp.tile([128, NB * T * G // 2], F32, name="sm_ps",
                                    tag="sm_ps", bufs=1)
                    off = half * (NB * T * G // 2)
                    HW2 = NB * T * G // 2
                    widths = []
                    o = 0
                    while o < HW2:
                        widths.append(min(512, HW2 - o))
                        o += widths[-1]
                    o = 0
                    for w in widths:
                        nc.tensor.matmul(
                            sm_ps[:, o:o + w], lt[0:T + 1, :],
                            selt[0:T + 1, off + o:off + o + w],
                            start=True, stop=True)
                        o += w
                    nc.scalar.add_instruction(
                        mybir.InstActivation(
                            name=nc.get_next_instruction_name(),
                            func=mybir.ActivationFunctionType.Reciprocal,
                            ins=[
                                nc.scalar.lower_ap(sm_ps[:]),
                                mybir.ImmediateValue(dtype=mybir.dt.float32, value=0.0),
                                mybir.ImmediateValue(dtype=mybir.dt.float32, value=1.0),
                                mybir.ImmediateValue(dtype=mybir.dt.float32, value=0.0),
                            ],
                            outs=[nc.scalar.lower_ap(
                                rcp[:].rearrange("p b t g -> p (b t g)")
                                [:, off:off + HW2])],
                        ))
                return rcp

            def phase_ab(n, rcp):
                base = n * 128 * T
                at = anch[:, n]  # [128, T, NCOL]

                def acol(k):
                    return (at[:, :, k].unsqueeze(1).unsqueeze(3)
                            .broadcast_to(FS))

                def gcol(k):
                    return gtb[:, :, k].unsqueeze(2).broadcast_to(FS)

                m1w = wp.tile(FS, F32, name="m1w", tag="m1w")
                nc.vector.tensor_tensor(out=m1w[:], in0=gcol(0), in1=acol(0), op=tt.min)
                m2w = wp.tile(FS, F32, name="m2w", tag="m2w")
                nc.vector.tensor_tensor(out=m2w[:], in0=gcol(1), in1=acol(1), op=tt.min)
                m1h = wp.tile(FS, F32, name="m1h", tag="m1h")
                nc.vector.tensor_tensor(out=m1h[:], in0=gcol(2), in1=acol(2), op=tt.min)
                m2h = wp.tile(FS, F32, name="m2h", tag="m2h")
                nc.vector.tensor_tensor(out=m2h[:], in0=gcol(3), in1=acol(3), op=tt.min)
                iwr = m1w
                nc.vector.tensor_tensor(out=iwr[:], in0=m1w[:], in1=m2w[:], op=tt.add)
                ihr = m1h
                nc.vector.tensor_tensor(out=ihr[:], in0=m1h[:], in1=m2h[:], op=tt.add)
                ihp = m2h
                nc.scalar.activation(ihp[:], ihr[:], mybir.ActivationFunctionType.Relu)
                inter = m2w
                nc.vector.scalar_tensor_tensor(
                    out=inter[:], in0=iwr[:], scalar=0.0, in1=ihp[:],
                    op0=tt.max, op1=tt.mult)
                r = inter
                nc.vector.tensor_tensor(out=r[:], in0=inter[:], in1=rcp[:], op=tt.mult)

                mxo = sp.tile([128, NB, T], F32, name="mxo", tag="mxo")
                nc.vector.reduce_max(out=mxo[:], in_=r[:], axis=mybir.AxisListType.X)
                eq32 = eq32s[n % 2]
                nc.vector.tensor_tensor(
                    out=eq32[:, :, :, 0:G], in0=r[:],
                    in1=mxo[:].unsqueeze(3).broadcast_to(FS), op=tt.is_equal)
                asn = sp.tile([128, NB, T, 5], F32, name="asn", tag="asn", bufs=2)
                for b in range(NB):
                    for ch in range(T // 4):
                        tp = pp.tile([128, 128], F32, name="tp", tag="tp")
                        nc.tensor.transpose(
                            tp[:],
                            eq32[:, b, 4 * ch:4 * ch + 4, :].rearrange(
                                "p t g -> p (t g)"),
                            iden[:])
                        eqT = wp.tile([128, 128], F32, name="eqT", tag="eqT")
                        nc.scalar.copy(eqT[:], tp[:])
                        pa = pp.tile([128, 4, 5], F32, name="pa", tag="pa")
                        nc.tensor.matmul(
                            pa[:].rearrange("p t q -> p (t q)"), eqT[:],
                            bdt[:, b], start=True, stop=True)
                        nc.scalar.copy(asn[:, b, 4 * ch:4 * ch + 4, :], pa[:])

                gmax = op.tile([128, NB, G], F32, name="gmax", tag="gmax")
                nc.vector.reduce_max(
                    out=gmax[:], in_=r[:].rearrange("p b t g -> p b g t"),
                    axis=mybir.AxisListType.X)
                nc.sync.dma_start(
                    out=aux_o[n].rearrange("p (b g) -> p b g", g=G), in_=gmax[:])
                return mxo, asn

            def phase_c(n, mxo, asn):
                base = n * 128 * T
                at = anch[:, n]

                pos = sp.tile([128, NB, T], F32, name="pos", tag="pos")
                nc.vector.tensor_scalar(
                    out=pos[:], in0=mxo[:], scalar1=POS_R, scalar2=None, op0=tt.is_ge)
                posi = sp.tile([128, NB, T], I32, name="posi", tag="posi")
                nc.scalar.copy(posi[:], pos[:])
                labf = sp.tile([128, NB, T], F32, name="labf", tag="labf")
                nc.vector.tensor_scalar(
                    out=labf[:], in0=mxo[:], scalar1=IGN_R, scalar2=-1.0,
                    op0=tt.is_ge, op1=tt.mult)
                nc.vector.copy_predicated(out=labf[:], mask=posi[:], data=asn[:, :, :, 4])
                labi = op.tile([128, NB, T], I32, name="labi", tag="labi")
                nc.scalar.copy(labi[:], labf[:])

                def acolB(k):
                    return at[:, :, k].unsqueeze(1).broadcast_to([128, NB, T])

                def acolB2(k0):
                    return (at[:, :, k0:k0 + 2].unsqueeze(1)
                            .broadcast_to([128, NB, T, 2]))

                rg = op.tile([128, NB, T, 4], F32, name="rg", tag="rg")
                nc.vector.tensor_tensor(
                    out=rg[:, :, :, 0:2], in0=asn[:, :, :, 0:2], in1=acolB2(5),
                    op=tt.subtract)
                nc.vector.tensor_tensor(
                    out=rg[:, :, :, 0:2], in0=rg[:, :, :, 0:2], in1=acolB2(7),
                    op=tt.mult)
                nc.vector.tensor_tensor(
                    out=rg[:, :, :, 2:4], in0=asn[:, :, :, 2:4], in1=acolB2(9),
                    op=tt.subtract)
                nc.vector.tensor_tensor(
                    out=rg[:], in0=rg[:],
                    in1=pos[:].unsqueeze(3).broadcast_to([128, NB, T, 4]), op=tt.mult)

                nc.sync.dma_start(
                    out=lab_o[:, base:base + 128 * T].rearrange(
                        "b (p t) -> p b t", p=128),
                    in_=labi[:])
                nc.sync.dma_start(
                    out=reg_o[:, base:base + 128 * T].rearrange(
                        "b (p t) c -> p b t c", p=128),
                    in_=rg[:])

            pend = []
            rcp_next = phase_rcp(0)
            for n in range(NT):
                rcp_cur = rcp_next
                if n + 1 < NT:
                    rcp_next = phase_rcp(n + 1)
                pend.append((n,) + phase_ab(n, rcp_cur))
                if len(pend) > 2:
                    phase_c(*pend.pop(0))
            for item in pend:
                phase_c(*item)

    from concourse.library_overlay import lower_extended_insts
    lower_extended_insts(nc)
    _split_multi_waits(nc)
    _BUILD_CACHE["nc"] = nc
    return nc


def _split_multi_waits(nc):
    """The TPB 64B instruction encoding has a single semaphore-wait slot.
    Tile's sem assignment can attach several waits to one instruction; walrus
    rejects those ("Too many sync wait commands"). Hoist all but one wait onto
    wait-only NoOps on the same engine immediately before the instruction."""
    import bass_rust

    for fn in nc.m.functions:
        for blk in fn.blocks:
            out = []
            for ins in blk.instructions:
                si = ins.sync_info
                if si is not None and si.on_wait and len(si.on_wait) > 1:
                    waits = list(si.on_wait)
                    for j, w in enumerate(waits[:-1]):
                        nop = bass_rust.InstNoOp(
                            name=f"{ins.name}-w{j}", engine=ins.engine,
                            ins=[], outs=[],
                            sync_info=mybir.SyncInfo(on_wait=[w], on_update=[]),
                        )
                        out.append(nop)
                    ins.sync_info = mybir.SyncInfo(
                        on_wait=[waits[-1]], on_update=list(si.on_update or []))
                out.append(ins)
            blk.instructions[:] = out


def _prep_anchor_ext(anchors):
    a = np.zeros((A_PAD, 4), np.float32)
    a[:A_TOT] = anchors
    a[A_TOT:, 0] = 1e8
    a[A_TOT:, 1] = 1e8
    a[A_TOT:, 2] = 1e8 + 10.0
    a[A_TOT:, 3] = 1e8 + 10.0
    x1, y1, x2, y2 = a[:, 0], a[:, 1], a[:, 2], a[:, 3]
    ew = x2 - x1 + 1.0
    eh = y2 - y1 + 1.0
    ext = np.zeros((A_PAD, NCOL), np.float32)
    ext[:, 0] = x2 + 1.0
    ext[:, 1] = -x1
    ext[:, 2] = y2 + 1.0
    ext[:, 3] = -y1
    ext[:, 4] = ew * eh
    ext[:, 5] = x1 + 0.5 * ew          # ecx
    ext[:, 6] = y1 + 0.5 * eh          # ecy
    ext[:, 7] = 1.0 / ew
    ext[:, 8] = 1.0 / eh
    ext[:, 9] = np.log(ew)
    ext[:, 10] = np.log(eh)
    ext[:, 11] = 1.0
    return ext


def _prep_gt_ext(bb, ids):
    # bb [n,G,4] f32, ids [n,G] -> [n, NCOL, G] f32
    n = bb.shape[0]
    x1, y1, x2, y2 = bb[..., 0], bb[..., 1], bb[..., 2], bb[..., 3]
    gw = x2 - x1 + 1.0
    gh = y2 - y1 + 1.0
    ext = np.zeros((n, NCOL, G), np.float32)
    ext[:, 0] = x2 + 1.0
    ext[:, 1] = -x1
    ext[:, 2] = y2 + 1.0
    ext[:, 3] = -y1
    ext[:, 4] = gw * gh
    ext[:, 5] = x1 + 0.5 * gw          # gcx
    ext[:, 6] = y1 + 0.5 * gh          # gcy
    ext[:, 7] = np.log(gw)
    ext[:, 8] = np.log(gh)
    ext[:, 9] = ids.astype(np.float32)
    return ext


def _exact_row_iou(anch_rows, gt_box):
    # anch_rows [T,4], gt_box [4] — float32, reference-order arithmetic
    one = np.float32(1.0)
    ax1, ay1, ax2, ay2 = (anch_rows[:, k] for k in range(4))
    gx1, gy1, gx2, gy2 = (np.float32(gt_box[k]) for k in range(4))
    iw = np.clip(np.minimum(ax2, gx2) - np.maximum(ax1, gx1) + one, 0.0, None)
    ih = np.clip(np.minimum(ay2, gy2) - np.maximum(ay1, gy1) + one, 0.0, None)
    area_a = (ax2 - ax1 + one) * (ay2 - ay1 + one)
    area_g = (gx2 - gx1 + one) * (gy2 - gy1 + one)
    inter = iw * ih
    return inter / (area_a + area_g - inter)


def kernel(bb_coord, bird_ids, anchors_l0, anchors_l1, anchors_l2, anchors_l3,
           anchors_l4, _trace=False):
    bb_coord = np.asarray(bb_coord, np.float32)
    bird_ids_np = np.asarray(bird_ids)
    anchors = np.concatenate(
        [np.asarray(x, np.float32) for x in
         (anchors_l0, anchors_l1, anchors_l2, anchors_l3, anchors_l4)], axis=0)

    anch_ext = _prep_anchor_ext(anchors)

    nc = _build_kernel()
    in_maps = []
    for c in range(N_CORES):
        bb = bb_coord[c * NB:(c + 1) * NB]
        ids = bird_ids_np[c * NB:(c + 1) * NB]
        gte = _prep_gt_ext(bb, ids)  # [NB, NCOL, G]
        gte_rep = np.broadcast_to(gte[:, None], (NB, 128, NCOL, G)).reshape(
            NB, 128, NCOL * G).copy()
        bdg = np.zeros((128, NB * G), np.float32)
        for bimg in range(NB):
            for tw in range(4):
                # rows tw*32+g, cols bimg*20 + tw*5 + q
                bdg[tw * 32:tw * 32 + G, bimg * G + tw * 5:bimg * G + tw * 5 + 5] = \
                    gte[bimg, 5:10, :].T
        selm = np.zeros((2 * T, NB * T * G), np.float32)
        col = 0
        for bimg in range(NB):
            for t in range(T):
                selm[t, col:col + G] = 1.0
                selm[T, col:col + G] = gte[bimg, 4, :]
                col += G
        in_maps.append({"anch": anch_ext, "gte": gte_rep, "bdg": bdg,
                        "iden": np.eye(128, dtype=np.float32), "sel": selm})

    if _trace:
        sys.path.insert(0, "/root/.axon_site")
        from trn_agent_boot.trn_boot import _ntff_profile_via_ctypes
        from antenv.axon_hooks import set_axon_ntff_profile_hook
        set_axon_ntff_profile_hook(
            _ntff_profile_via_ctypes("/opt/axon/libaxon_pjrt.so"))
    res = run_bass_kernel_spmd(nc, in_maps, core_ids=list(range(N_CORES)),
                               trace=_trace)
    outs = res.results

    labels = np.zeros((B, A_TOT), np.int32)
    reg = np.zeros((B, A_TOT, 4), np.float32)
    one = np.float32(1.0)
    half = np.float32(0.5)
    for c in range(N_CORES):
        o = outs[c]
        labels[c * NB:(c + 1) * NB] = o["labels"][:, :A_TOT]
        reg[c * NB:(c + 1) * NB] = o["reg"][:, :A_TOT]
        aux = o["aux"].reshape(NT, 128, NB, G)
        for bi in range(NB):
            b = c * NB + bi
            for g in range(G):
                m = aux[:, :, bi, g]                     # [NT, 128]
                k = int(m.argmax())                      # first-index ties
                nstar, pstar = divmod(k, 128)
                base = nstar * 128 * T + pstar * T
                arow = np.empty((T, 4), np.float32)
                hi = min(base + T, A_TOT)
                arow[:hi - base] = anchors[base:hi]
                if hi - base < T:
                    arow[hi - base:] = [1e8, 1e8, 1e8 + 10.0, 1e8 + 10.0]
                ov = _exact_row_iou(arow, bb_coord[b, g])
                tstar = int(ov.argmax())
                a = base + tstar
                if a >= A_TOT:
                    continue
                ex = anchors[a]
                gtb = bb_coord[b, g]
                ew = ex[2] - ex[0] + one
                eh = ex[3] - ex[1] + one
                ecx = ex[0] + half * ew
                ecy = ex[1] + half * eh
                gw = gtb[2] - gtb[0] + one
                gh = gtb[3] - gtb[1] + one
                gcx = gtb[0] + half * gw
                gcy = gtb[1] + half * gh
                labels[b, a] = np.int32(bird_ids_np[b, g])
                reg[b, a, 0] = (gcx - ecx) / ew
                reg[b, a, 1] = (gcy - ecy) / eh
                reg[b, a, 2] = np.log(gw / ew)
                reg[b, a, 3] = np.log(gh / eh)
    if _trace:
        return (labels, reg), res
    return labels, reg


# revision 32
# speedup vs baseline: 1.2791x; 1.2791x over previous
"""AnchorTargetLayer Trainium2 kernel.

Data-parallel over batch: 32 images / 8 NeuronCores = 4 images per core.
All 5 anchor levels are concatenated into one 65472-anchor axis (padded to
65536) — the per-level structure only matters for the forced-positive
(per-GT best anchor) rule, and argmax-over-levels of per-level maxima is
identical to a single global argmax over the concatenated axis.

Per 128x16-anchor tile, all 4 images are processed in one instruction
(free size 4*16*20). Ordering/thresholds use r = inter/(area_a+area_g),
a strictly monotone transform of IoU (r = ov/(1+ov)), so no division by
the union is needed; thresholds 0.4/0.5 become 2/7 and 1/3.

Engine split: GpSimd takes the min/add front half of the IoU, ScalarE the
relus, DVE the multiplies/reduces/compares and the one-hot gather of
(gcx, gcy, log gw, log gh, id) for assigned GTs.

The per-GT argmax over anchors is resolved on host: the device emits the
per-(tile, partition) max of r for each (image, gt); the host picks the
winning row (first-index ties, matching the reference) and recomputes the
16 IoUs of that row exactly to find the winning anchor, then applies the
20-per-image forced-positive patches.
"""

import sys

import numpy as np

sys.path.insert(0, "/opt/trn_rl_repo")

import concourse.bass as bass
import concourse.mybir as mybir
from concourse.bass_utils import run_bass_kernel_spmd
from concourse.tile import TileContext

F32 = mybir.dt.float32
I32 = mybir.dt.int32

N_CORES = 8
B = 32
G = 20
NB = B // N_CORES          # images per core
A_TOT = 65472              # 49152 + 12288 + 3072 + 768 + 192
A_PAD = 65536
T = 32                     # anchors per partition per tile
NT = A_PAD // (128 * T)    # 16 tiles
NCOL = 13                  # anchor-ext columns
POS_R = float(np.float32(1.0 / 3.0))   # ov >= 0.5  <=>  r >= 1/3
IGN_R = float(np.float32(2.0 / 7.0))   # ov >= 0.4  <=>  r >= 2/7

_BUILD_CACHE = {}


def _build_kernel():
    if "nc" in _BUILD_CACHE:
        return _BUILD_CACHE["nc"]
    nc = bass.Bass()
    an = nc.declare_dram_parameter("anch", [A_PAD, NCOL], F32, isOutput=False)
    gt = nc.declare_dram_parameter("gte", [NB, 128, NCOL * G], F32, isOutput=False)
    bd = nc.declare_dram_parameter("bdg", [128, NB * G], F32, isOutput=False)
    idn = nc.declare_dram_parameter("iden", [128, 128], F32, isOutput=False)
    sel = nc.declare_dram_parameter("sel", [2 * T, NB * T * G], F32, isOutput=False)
    lab_o = nc.declare_dram_parameter("labels", [NB, A_PAD], I32, isOutput=True)
    reg_o = nc.declare_dram_parameter("reg", [NB, A_PAD, 4], F32, isOutput=True)
    aux_o = nc.declare_dram_parameter("aux", [NT, 128, NB * G], F32, isOutput=True)

    tt = mybir.AluOpType
    FS = [128, NB, T, G]

    with TileContext(nc) as tc:
        with (
            tc.tile_pool(name="res", bufs=1) as rp,
            tc.tile_pool(name="wk", bufs=2) as wp,
            tc.tile_pool(name="sm", bufs=3) as sp,
            tc.tile_pool(name="ot", bufs=3) as op,
            tc.tile_pool(name="ps", bufs=2, space="PSUM") as pp,
        ):
            anch = rp.tile([128, NT, T, NCOL], F32, tag="anch")
            for n in range(NT):
                nc.sync.dma_start(
                    out=anch[:, n],
                    in_=an[n * 128 * T:(n + 1) * 128 * T].rearrange(
                        "(p t) c -> p t c", p=128),
                )
            gtb = rp.tile([128, NB, NCOL, G], F32, tag="gtb")
            for b in range(NB):
                nc.sync.dma_start(
                    out=gtb[:, b],
                    in_=gt[b].rearrange("p (c g) -> p c g", g=G),
                )
            bdt = rp.tile([128, NB, G], F32, tag="bdt")
            nc.sync.dma_start(out=bdt[:], in_=bd[:].rearrange("p (b g) -> p b g", g=G))
            iden = rp.tile([128, 128], F32, tag="iden")
            nc.sync.dma_start(out=iden[:], in_=idn[:])
            selt = rp.tile([2 * T, NB * T * G], F32, tag="selt")
            nc.sync.dma_start(out=selt[:], in_=sel[:])
            eq32s = [rp.tile([128, NB, T, 32], F32, name=f"eq32_{i}", tag=f"eq32_{i}")
                     for i in range(2)]
            for e in eq32s:
                nc.vector.memset(e[:], 0.0)

            def phase_rcp(n):
                at = anch[:, n]
                ltin = wp.tile([128, 2, T], F32, name="ltin", tag="ltin")
                nc.scalar.copy(ltin[:], at[:, :, 4:13:7].rearrange("p t c -> p c t"))
                lt_ps = pp.tile([2 * T, 128], F32, name="lt_ps", tag="lt_ps", bufs=1)
                nc.tensor.transpose(
                    lt_ps[:], ltin[:].rearrange("p c t -> p (c t)"), iden[:])
                lt = wp.tile([2 * T, 128], F32, name="lt", tag="lt")
                nc.scalar.copy(lt[:], lt_ps[:])
                rcp = wp.tile(FS, F32, name="rcp", tag="rcp")
                for half in range(2):
                    sm_ps = pp.tile([128, NB * T * G // 2], F32, name="sm_ps",
                                    tag="sm_ps", bufs=1)
                    off = half * (NB * T * G // 2)
                    HW2 = NB * T * G // 2
                    widths = []
                    o = 0
                    while o < HW2:
                        widths.append(min(512, HW2 - o))
                        o += widths[-1]
                    o = 0
                    for w in widths:
                        nc.tensor.matmul(
                            sm_ps[:, o:o + w], lt[0:T + 1, :],
                            selt[0:T + 1, off + o:off + o + w],
                            start=True, stop=True)
                        o += w
                    nc.scalar.add_instruction(
                        mybir.InstActivation(
                            name=nc.get_next_instruction_name(),
                            func=mybir.ActivationFunctionType.Reciprocal,
                            ins=[
                                nc.scalar.lower_ap(sm_ps[:]),
                                mybir.ImmediateValue(dtype=mybir.dt.float32, value=0.0),
                                mybir.ImmediateValue(dtype=mybir.dt.float32, value=1.0),
                                mybir.ImmediateValue(dtype=mybir.dt.float32, value=0.0),
                            ],
                            outs=[nc.scalar.lower_ap(
                                rcp[:].rearrange("p b t g -> p (b t g)")
                                [:, off:off + HW2])],
                        ))
                return rcp

            def phase_ab(n, rcp):
                base = n * 128 * T
                at = anch[:, n]  # [128, T, NCOL]

                def acol(k):
                    return (at[:, :, k].unsqueeze(1).unsqueeze(3)
                            .broadcast_to(FS))

                def gcol(k):
                    return gtb[:, :, k].unsqueeze(2).broadcast_to(FS)

                m1w = wp.tile(FS, F32, name="m1w", tag="m1w")
                nc.vector.tensor_tensor(out=m1w[:], in0=gcol(0), in1=acol(0), op=tt.min)
                m2w = wp.tile(FS, F32, name="m2w", tag="m2w")
                nc.vector.tensor_tensor(out=m2w[:], in0=gcol(1), in1=acol(1), op=tt.min)
                m1h = wp.tile(FS, F32, name="m1h", tag="m1h")
                nc.vector.tensor_tensor(out=m1h[:], in0=gcol(2), in1=acol(2), op=tt.min)
                m2h = wp.tile(FS, F32, name="m2h", tag="m2h")
                nc.vector.tensor_tensor(out=m2h[:], in0=gcol(3), in1=acol(3), op=tt.min)
                iwr = m1w
                nc.vector.tensor_tensor(out=iwr[:], in0=m1w[:], in1=m2w[:], op=tt.add)
                ihr = m1h
                nc.vector.tensor_tensor(out=ihr[:], in0=m1h[:], in1=m2h[:], op=tt.add)
                ihp = m2h
                nc.vector.tensor_scalar(
                    out=ihp[:], in0=ihr[:], scalar1=0.0, scalar2=None, op0=tt.max)
                inter = m2w
                nc.vector.scalar_tensor_tensor(
                    out=inter[:], in0=iwr[:], scalar=0.0, in1=ihp[:],
                    op0=tt.max, op1=tt.mult)
                r = inter
                nc.vector.tensor_tensor(out=r[:], in0=inter[:], in1=rcp[:], op=tt.mult)

                mxo = sp.tile([128, NB, T], F32, name="mxo", tag="mxo")
                nc.vector.reduce_max(out=mxo[:], in_=r[:], axis=mybir.AxisListType.X)
                eq32 = eq32s[n % 2]
                nc.vector.tensor_tensor(
                    out=eq32[:, :, :, 0:G], in0=r[:],
                    in1=mxo[:].unsqueeze(3).broadcast_to(FS), op=tt.is_equal)
                asn = sp.tile([128, NB, T, 5], F32, name="asn", tag="asn", bufs=2)
                for b in range(NB):
                    for ch in range(T // 4):
                        tp = pp.tile([128, 128], F32, name="tp", tag="tp")
                        nc.tensor.transpose(
                            tp[:],
                            eq32[:, b, 4 * ch:4 * ch + 4, :].rearrange(
                                "p t g -> p (t g)"),
                            iden[:])
                        eqT = wp.tile([128, 128], F32, name="eqT", tag="eqT")
                        nc.scalar.copy(eqT[:], tp[:])
                        pa = pp.tile([128, 4, 5], F32, name="pa", tag="pa")
                        nc.tensor.matmul(
                            pa[:].rearrange("p t q -> p (t q)"), eqT[:],
                            bdt[:, b], start=True, stop=True)
                        nc.scalar.copy(asn[:, b, 4 * ch:4 * ch + 4, :], pa[:])

                gmax = op.tile([128, NB, G], F32, name="gmax", tag="gmax")
                nc.vector.reduce_max(
                    out=gmax[:], in_=r[:].rearrange("p b t g -> p b g t"),
                    axis=mybir.AxisListType.X)
                nc.sync.dma_start(
                    out=aux_o[n].rearrange("p (b g) -> p b g", g=G), in_=gmax[:])
                return mxo, asn

            def phase_c(n, mxo, asn):
                base = n * 128 * T
                at = anch[:, n]

                pos = sp.tile([128, NB, T], F32, name="pos", tag="pos")
                nc.vector.tensor_scalar(
                    out=pos[:], in0=mxo[:], scalar1=POS_R, scalar2=None, op0=tt.is_ge)
                posi = sp.tile([128, NB, T], I32, name="posi", tag="posi")
                nc.scalar.copy(posi[:], pos[:])
                labf = sp.tile([128, NB, T], F32, name="labf", tag="labf")
                nc.vector.tensor_scalar(
                    out=labf[:], in0=mxo[:], scalar1=IGN_R, scalar2=-1.0,
                    op0=tt.is_ge, op1=tt.mult)
                nc.vector.copy_predicated(out=labf[:], mask=posi[:], data=asn[:, :, :, 4])
                labi = op.tile([128, NB, T], I32, name="labi", tag="labi")
                nc.scalar.copy(labi[:], labf[:])

                def acolB(k):
                    return at[:, :, k].unsqueeze(1).broadcast_to([128, NB, T])

                def acolB2(k0):
                    return (at[:, :, k0:k0 + 2].unsqueeze(1)
                            .broadcast_to([128, NB, T, 2]))

                rg = op.tile([128, NB, T, 4], F32, name="rg", tag="rg")
                nc.vector.tensor_tensor(
                    out=rg[:, :, :, 0:2], in0=asn[:, :, :, 0:2], in1=acolB2(5),
                    op=tt.subtract)
                nc.vector.tensor_tensor(
                    out=rg[:, :, :, 0:2], in0=rg[:, :, :, 0:2], in1=acolB2(7),
                    op=tt.mult)
                nc.vector.tensor_tensor(
                    out=rg[:, :, :, 2:4], in0=asn[:, :, :, 2:4], in1=acolB2(9),
                    op=tt.subtract)
                nc.vector.tensor_tensor(
                    out=rg[:], in0=rg[:],
                    in1=pos[:].unsqueeze(3).broadcast_to([128, NB, T, 4]), op=tt.mult)

                nc.sync.dma_start(
                    out=lab_o[:, base:base + 128 * T].rearrange(
                        "b (p t) -> p b t", p=128),
                    in_=labi[:])
                nc.sync.dma_start(
                    out=reg_o[:, base:base + 128 * T].rearrange(
                        "b (p t) c -> p b t c", p=128),
                    in_=rg[:])

            pend = []
            rcp_next = phase_rcp(0)
            for n in range(NT):
                rcp_cur = rcp_next
                if n + 1 < NT:
                    rcp_next = phase_rcp(n + 1)
                pend.append((n,) + phase_ab(n, rcp_cur))
                if len(pend) > 2:
                    phase_c(*pend.pop(0))
            for item in pend:
                phase_c(*item)

    from concourse.library_overlay import lower_extended_insts
    lower_extended_insts(nc)
    _split_multi_waits(nc)
    _BUILD_CACHE["nc"] = nc
    return nc


def _split_multi_waits(nc):
    """The TPB 64B instruction encoding has a single semaphore-wait slot.
    Tile's sem assignment can attach several waits to one instruction; walrus
    rejects those ("Too many sync wait commands"). Hoist all but one wait onto
    wait-only NoOps on the same engine immediately before the instruction."""
    import bass_rust

    for fn in nc.m.functions:
        for blk in fn.blocks:
            out = []
            for ins in blk.instructions:
                si = ins.sync_info
                if si is not None and si.on_wait and len(si.on_wait) > 1:
                    waits = list(si.on_wait)
                    for j, w in enumerate(waits[:-1]):
                        nop = bass_rust.InstNoOp(
                            name=f"{ins.name}-w{j}", engine=ins.engine,
                            ins=[], outs=[],
                            sync_info=mybir.SyncInfo(on_wait=[w], on_update=[]),
                        )
                        out.append(nop)
                    ins.sync_info = mybir.SyncInfo(
                        on_wait=[waits[-1]], on_update=list(si.on_update or []))
                out.append(ins)
            blk.instructions[:] = out


def _prep_anchor_ext(anchors):
    a = np.zeros((A_PAD, 4), np.float32)
    a[:A_TOT] = anchors
    a[A_TOT:, 0] = 1e8
    a[A_TOT:, 1] = 1e8
    a[A_TOT:, 2] = 1e8 + 10.0
    a[A_TOT:, 3] = 1e8 + 10.0
    x1, y1, x2, y2 = a[:, 0], a[:, 1], a[:, 2], a[:, 3]
    ew = x2 - x1 + 1.0
    eh = y2 - y1 + 1.0
    ext = np.zeros((A_PAD, NCOL), np.float32)
    ext[:, 0] = x2 + 1.0
    ext[:, 1] = -x1
    ext[:, 2] = y2 + 1.0
    ext[:, 3] = -y1
    ext[:, 4] = ew * eh
    ext[:, 5] = x1 + 0.5 * ew          # ecx
    ext[:, 6] = y1 + 0.5 * eh          # ecy
    ext[:, 7] = 1.0 / ew
    ext[:, 8] = 1.0 / eh
    ext[:, 9] = np.log(ew)
    ext[:, 10] = np.log(eh)
    ext[:, 11] = 1.0
    return ext


def _prep_gt_ext(bb, ids):
    # bb [n,G,4] f32, ids [n,G] -> [n, NCOL, G] f32
    n = bb.shape[0]
    x1, y1, x2, y2 = bb[..., 0], bb[..., 1], bb[..., 2], bb[..., 3]
    gw = x2 - x1 + 1.0
    gh = y2 - y1 + 1.0
    ext = np.zeros((n, NCOL, G), np.float32)
    ext[:, 0] = x2 + 1.0
    ext[:, 1] = -x1
    ext[:, 2] = y2 + 1.0
    ext[:, 3] = -y1
    ext[:, 4] = gw * gh
    ext[:, 5] = x1 + 0.5 * gw          # gcx
    ext[:, 6] = y1 + 0.5 * gh          # gcy
    ext[:, 7] = np.log(gw)
    ext[:, 8] = np.log(gh)
    ext[:, 9] = ids.astype(np.float32)
    return ext


def _exact_row_iou(anch_rows, gt_box):
    # anch_rows [T,4], gt_box [4] — float32, reference-order arithmetic
    one = np.float32(1.0)
    ax1, ay1, ax2, ay2 = (anch_rows[:, k] for k in range(4))
    gx1, gy1, gx2, gy2 = (np.float32(gt_box[k]) for k in range(4))
    iw = np.clip(np.minimum(ax2, gx2) - np.maximum(ax1, gx1) + one, 0.0, None)
    ih = np.clip(np.minimum(ay2, gy2) - np.maximum(ay1, gy1) + one, 0.0, None)
    area_a = (ax2 - ax1 + one) * (ay2 - ay1 + one)
    area_g = (gx2 - gx1 + one) * (gy2 - gy1 + one)
    inter = iw * ih
    return inter / (area_a + area_g - inter)


def kernel(bb_coord, bird_ids, anchors_l0, anchors_l1, anchors_l2, anchors_l3,
           anchors_l4, _trace=False):
    bb_coord = np.asarray(bb_coord, np.float32)
    bird_ids_np = np.asarray(bird_ids)
    anchors = np.concatenate(
        [np.asarray(x, np.float32) for x in
         (anchors_l0, anchors_l1, anchors_l2, anchors_l3, anchors_l4)], axis=0)

    anch_ext = _prep_anchor_ext(anchors)

    nc = _build_kernel()
    in_maps = []
    for c in range(N_CORES):
        bb = bb_coord[c * NB:(c + 1) * NB]
        ids = bird_ids_np[c * NB:(c + 1) * NB]
        gte = _prep_gt_ext(bb, ids)  # [NB, NCOL, G]
        gte_rep = np.broadcast_to(gte[:, None], (NB, 128, NCOL, G)).reshape(
            NB, 128, NCOL * G).copy()
        bdg = np.zeros((128, NB * G), np.float32)
        for bimg in range(NB):
            for tw in range(4):
                # rows tw*32+g, cols bimg*20 + tw*5 + q
                bdg[tw * 32:tw * 32 + G, bimg * G + tw * 5:bimg * G + tw * 5 + 5] = \
                    gte[bimg, 5:10, :].T
        selm = np.zeros((2 * T, NB * T * G), np.float32)
        col = 0
        for bimg in range(NB):
            for t in range(T):
                selm[t, col:col + G] = 1.0
                selm[T, col:col + G] = gte[bimg, 4, :]
                col += G
        in_maps.append({"anch": anch_ext, "gte": gte_rep, "bdg": bdg,
                        "iden": np.eye(128, dtype=np.float32), "sel": selm})

    if _trace:
        sys.path.insert(0, "/root/.axon_site")
        from trn_agent_boot.trn_boot import _ntff_profile_via_ctypes
        from antenv.axon_hooks import set_axon_ntff_profile_hook
        set_axon_ntff_profile_hook(
            _ntff_profile_via_ctypes("/opt/axon/libaxon_pjrt.so"))
    res = run_bass_kernel_spmd(nc, in_maps, core_ids=list(range(N_CORES)),
                               trace=_trace)
    outs = res.results

    labels = np.zeros((B, A_TOT), np.int32)
    reg = np.zeros((B, A_TOT, 4), np.float32)
    one = np.float32(1.0)
    half = np.float32(0.5)
    for c in range(N_CORES):
        o = outs[c]
        labels[c * NB:(c + 1) * NB] = o["labels"][:, :A_TOT]
        reg[c * NB:(c + 1) * NB] = o["reg"][:, :A_TOT]
        aux = o["aux"].reshape(NT, 128, NB, G)
        for bi in range(NB):
            b = c * NB + bi
            for g in range(G):
                m = aux[:, :, bi, g]                     # [NT, 128]
                k = int(m.argmax())                      # first-index ties
                nstar, pstar = divmod(k, 128)
                base = nstar * 128 * T + pstar * T
                arow = np.empty((T, 4), np.float32)
                hi = min(base + T, A_TOT)
                arow[:hi - base] = anchors[base:hi]
                if hi - base < T:
                    arow[hi - base:] = [1e8, 1e8, 1e8 + 10.0, 1e8 + 10.0]
                ov = _exact_row_iou(arow, bb_coord[b, g])
                tstar = int(ov.argmax())
                a = base + tstar
                if a >= A_TOT:
                    continue
                ex = anchors[a]
                gtb = bb_coord[b, g]
                ew = ex[2] - ex[0] + one
                eh = ex[3] - ex[1] + one
                ecx = ex[0] + half * ew
                ecy = ex[1] + half * eh
                gw = gtb[2] - gtb[0] + one
                gh = gtb[3] - gtb[1] + one
                gcx = gtb[0] + half * gw
                gcy = gtb[1] + half * gh
                labels[b, a] = np.int32(bird_ids_np[b, g])
                reg[b, a, 0] = (gcx - ecx) / ew
                reg[b, a, 1] = (gcy - ecy) / eh
                reg[b, a, 2] = np.log(gw / ew)
                reg[b, a, 3] = np.log(gh / eh)
    if _trace:
        return (labels, reg), res
    return labels, reg


# revision 33
# speedup vs baseline: 1.3625x; 1.0652x over previous
"""AnchorTargetLayer Trainium2 kernel.

Data-parallel over batch: 32 images / 8 NeuronCores = 4 images per core.
All 5 anchor levels are concatenated into one 65472-anchor axis (padded to
65536) — the per-level structure only matters for the forced-positive
(per-GT best anchor) rule, and argmax-over-levels of per-level maxima is
identical to a single global argmax over the concatenated axis.

Per 128x16-anchor tile, all 4 images are processed in one instruction
(free size 4*16*20). Ordering/thresholds use r = inter/(area_a+area_g),
a strictly monotone transform of IoU (r = ov/(1+ov)), so no division by
the union is needed; thresholds 0.4/0.5 become 2/7 and 1/3.

Engine split: GpSimd takes the min/add front half of the IoU, ScalarE the
relus, DVE the multiplies/reduces/compares and the one-hot gather of
(gcx, gcy, log gw, log gh, id) for assigned GTs.

The per-GT argmax over anchors is resolved on host: the device emits the
per-(tile, partition) max of r for each (image, gt); the host picks the
winning row (first-index ties, matching the reference) and recomputes the
16 IoUs of that row exactly to find the winning anchor, then applies the
20-per-image forced-positive patches.
"""

import sys

import numpy as np

sys.path.insert(0, "/opt/trn_rl_repo")

import concourse.bass as bass
import concourse.mybir as mybir
from concourse.bass_utils import run_bass_kernel_spmd
from concourse.tile import TileContext

F32 = mybir.dt.float32
I32 = mybir.dt.int32

N_CORES = 8
B = 32
G = 20
NB = B // N_CORES          # images per core
A_TOT = 65472              # 49152 + 12288 + 3072 + 768 + 192
A_PAD = 65536
T = 32                     # anchors per partition per tile
NT = A_PAD // (128 * T)    # 16 tiles
NCOL = 13                  # anchor-ext columns
POS_R = float(np.float32(1.0 / 3.0))   # ov >= 0.5  <=>  r >= 1/3
IGN_R = float(np.float32(2.0 / 7.0))   # ov >= 0.4  <=>  r >= 2/7

_BUILD_CACHE = {}


def _build_kernel():
    if "nc" in _BUILD_CACHE:
        return _BUILD_CACHE["nc"]
    nc = bass.Bass()
    an = nc.declare_dram_parameter("anch", [A_PAD, NCOL], F32, isOutput=False)
    gt = nc.declare_dram_parameter("gte", [NB, 128, NCOL * G], F32, isOutput=False)
    bd = nc.declare_dram_parameter("bdg", [128, NB * G], F32, isOutput=False)
    idn = nc.declare_dram_parameter("iden", [128, 128], F32, isOutput=False)
    sel = nc.declare_dram_parameter("sel", [2 * T, NB * T * G], F32, isOutput=False)
    lab_o = nc.declare_dram_parameter("labels", [NB, A_PAD], I32, isOutput=True)
    reg_o = nc.declare_dram_parameter("reg", [NB, A_PAD, 4], F32, isOutput=True)
    aux_o = nc.declare_dram_parameter("aux", [NT, 128, NB * G], F32, isOutput=True)

    tt = mybir.AluOpType
    FS = [128, NB, T, G]

    with TileContext(nc) as tc:
        with (
            tc.tile_pool(name="res", bufs=1) as rp,
            tc.tile_pool(name="wk", bufs=2) as wp,
            tc.tile_pool(name="sm", bufs=3) as sp,
            tc.tile_pool(name="ot", bufs=3) as op,
            tc.tile_pool(name="ps", bufs=2, space="PSUM") as pp,
        ):
            anch = rp.tile([128, NT, T, NCOL], F32, tag="anch")
            for n in range(NT):
                nc.sync.dma_start(
                    out=anch[:, n],
                    in_=an[n * 128 * T:(n + 1) * 128 * T].rearrange(
                        "(p t) c -> p t c", p=128),
                )
            gtb = rp.tile([128, NB, NCOL, G], F32, tag="gtb")
            for b in range(NB):
                nc.sync.dma_start(
                    out=gtb[:, b],
                    in_=gt[b].rearrange("p (c g) -> p c g", g=G),
                )
            bdt = rp.tile([128, NB, G], F32, tag="bdt")
            nc.sync.dma_start(out=bdt[:], in_=bd[:].rearrange("p (b g) -> p b g", g=G))
            iden = rp.tile([128, 128], F32, tag="iden")
            nc.sync.dma_start(out=iden[:], in_=idn[:])
            selt = rp.tile([2 * T, NB * T * G], F32, tag="selt")
            nc.sync.dma_start(out=selt[:], in_=sel[:])
            eq32s = [rp.tile([128, NB, T, 32], F32, name=f"eq32_{i}", tag=f"eq32_{i}")
                     for i in range(2)]
            for e in eq32s:
                nc.vector.memset(e[:], 0.0)

            def phase_rcp(n):
                at = anch[:, n]
                ltin = wp.tile([128, 2, T], F32, name="ltin", tag="ltin")
                nc.scalar.copy(ltin[:], at[:, :, 4:13:7].rearrange("p t c -> p c t"))
                lt_ps = pp.tile([2 * T, 128], F32, name="lt_ps", tag="lt_ps", bufs=1)
                nc.tensor.transpose(
                    lt_ps[:], ltin[:].rearrange("p c t -> p (c t)"), iden[:])
                lt = wp.tile([2 * T, 128], F32, name="lt", tag="lt")
                nc.scalar.copy(lt[:], lt_ps[:])
                rcp = wp.tile(FS, F32, name="rcp", tag="rcp")
                for half in range(2):
                    sm_ps = pp.tile([128, NB * T * G // 2], F32, name="sm_ps",
                                    tag="sm_ps", bufs=1)
                    off = half * (NB * T * G // 2)
                    HW2 = NB * T * G // 2
                    widths = []
                    o = 0
                    while o < HW2:
                        widths.append(min(512, HW2 - o))
                        o += widths[-1]
                    o = 0
                    for w in widths:
                        nc.tensor.matmul(
                            sm_ps[:, o:o + w], lt[0:T + 1, :],
                            selt[0:T + 1, off + o:off + o + w],
                            start=True, stop=True)
                        o += w
                    nc.scalar.add_instruction(
                        mybir.InstActivation(
                            name=nc.get_next_instruction_name(),
                            func=mybir.ActivationFunctionType.Reciprocal,
                            ins=[
                                nc.scalar.lower_ap(sm_ps[:]),
                                mybir.ImmediateValue(dtype=mybir.dt.float32, value=0.0),
                                mybir.ImmediateValue(dtype=mybir.dt.float32, value=1.0),
                                mybir.ImmediateValue(dtype=mybir.dt.float32, value=0.0),
                            ],
                            outs=[nc.scalar.lower_ap(
                                rcp[:].rearrange("p b t g -> p (b t g)")
                                [:, off:off + HW2])],
                        ))
                return rcp

            def phase_ab(n, rcp):
                base = n * 128 * T
                at = anch[:, n]  # [128, T, NCOL]

                def acol(k):
                    return (at[:, :, k].unsqueeze(1).unsqueeze(3)
                            .broadcast_to(FS))

                def gcol(k):
                    return gtb[:, :, k].unsqueeze(2).broadcast_to(FS)

                m1w = wp.tile(FS, F32, name="m1w", tag="m1w")
                nc.vector.tensor_tensor(out=m1w[:], in0=gcol(0), in1=acol(0), op=tt.min)
                m2w = wp.tile(FS, F32, name="m2w", tag="m2w")
                nc.vector.tensor_tensor(out=m2w[:], in0=gcol(1), in1=acol(1), op=tt.min)
                m1h = wp.tile(FS, F32, name="m1h", tag="m1h")
                nc.vector.tensor_tensor(out=m1h[:], in0=gcol(2), in1=acol(2), op=tt.min)
                m2h = wp.tile(FS, F32, name="m2h", tag="m2h")
                nc.vector.tensor_tensor(out=m2h[:], in0=gcol(3), in1=acol(3), op=tt.min)
                iwr = m1w
                nc.vector.tensor_tensor(out=iwr[:], in0=m1w[:], in1=m2w[:], op=tt.add)
                ihr = m1h
                nc.vector.tensor_tensor(out=ihr[:], in0=m1h[:], in1=m2h[:], op=tt.add)
                ihp = m2h
                nc.vector.tensor_scalar(
                    out=ihp[:], in0=ihr[:], scalar1=0.0, scalar2=None, op0=tt.max)
                inter = m2w
                nc.vector.scalar_tensor_tensor(
                    out=inter[:], in0=iwr[:], scalar=0.0, in1=ihp[:],
                    op0=tt.max, op1=tt.mult)
                r = inter
                nc.vector.tensor_tensor(out=r[:], in0=inter[:], in1=rcp[:], op=tt.mult)

                mxo = sp.tile([128, NB, T], F32, name="mxo", tag="mxo")
                nc.vector.reduce_max(out=mxo[:], in_=r[:], axis=mybir.AxisListType.X)
                eq32 = eq32s[n % 2]
                nc.vector.tensor_tensor(
                    out=eq32[:, :, :, 0:G], in0=r[:],
                    in1=mxo[:].unsqueeze(3).broadcast_to(FS), op=tt.is_equal)
                asn = sp.tile([128, NB, T, 5], F32, name="asn", tag="asn", bufs=2)
                for b in range(NB):
                    for ch in range(T // 4):
                        tp = pp.tile([128, 128], F32, name="tp", tag="tp")
                        nc.tensor.transpose(
                            tp[:],
                            eq32[:, b, 4 * ch:4 * ch + 4, :].rearrange(
                                "p t g -> p (t g)"),
                            iden[:])
                        eqT = wp.tile([128, 128], F32, name="eqT", tag="eqT")
                        nc.scalar.copy(eqT[:], tp[:])
                        pa = pp.tile([128, 4, 5], F32, name="pa", tag="pa")
                        nc.tensor.matmul(
                            pa[:].rearrange("p t q -> p (t q)"), eqT[:],
                            bdt[:, b], start=True, stop=True)
                        nc.scalar.copy(asn[:, b, 4 * ch:4 * ch + 4, :], pa[:])

                gmax = op.tile([128, NB, G], F32, name="gmax", tag="gmax")
                nc.vector.reduce_max(
                    out=gmax[:], in_=r[:].rearrange("p b t g -> p b g t"),
                    axis=mybir.AxisListType.X)
                nc.sync.dma_start(
                    out=aux_o[n].rearrange("p (b g) -> p b g", g=G), in_=gmax[:])
                return mxo, asn

            def phase_c(n, mxo, asn):
                base = n * 128 * T
                at = anch[:, n]

                pos = sp.tile([128, NB, T], F32, name="pos", tag="pos")
                nc.vector.tensor_scalar(
                    out=pos[:], in0=mxo[:], scalar1=POS_R, scalar2=None, op0=tt.is_ge)
                posi = sp.tile([128, NB, T], I32, name="posi", tag="posi")
                nc.vector.tensor_copy(out=posi[:], in_=pos[:])
                labf = sp.tile([128, NB, T], F32, name="labf", tag="labf")
                nc.vector.tensor_scalar(
                    out=labf[:], in0=mxo[:], scalar1=IGN_R, scalar2=-1.0,
                    op0=tt.is_ge, op1=tt.mult)
                nc.vector.copy_predicated(out=labf[:], mask=posi[:], data=asn[:, :, :, 4])
                labi = op.tile([128, NB, T], I32, name="labi", tag="labi")
                nc.vector.tensor_copy(out=labi[:], in_=labf[:])

                def acolB(k):
                    return at[:, :, k].unsqueeze(1).broadcast_to([128, NB, T])

                def acolB2(k0):
                    return (at[:, :, k0:k0 + 2].unsqueeze(1)
                            .broadcast_to([128, NB, T, 2]))

                rg = op.tile([128, NB, T, 4], F32, name="rg", tag="rg")
                nc.vector.tensor_tensor(
                    out=rg[:, :, :, 0:2], in0=asn[:, :, :, 0:2], in1=acolB2(5),
                    op=tt.subtract)
                nc.vector.tensor_tensor(
                    out=rg[:, :, :, 0:2], in0=rg[:, :, :, 0:2], in1=acolB2(7),
                    op=tt.mult)
                nc.vector.tensor_tensor(
                    out=rg[:, :, :, 2:4], in0=asn[:, :, :, 2:4], in1=acolB2(9),
                    op=tt.subtract)
                nc.vector.tensor_tensor(
                    out=rg[:], in0=rg[:],
                    in1=pos[:].unsqueeze(3).broadcast_to([128, NB, T, 4]), op=tt.mult)

                nc.sync.dma_start(
                    out=lab_o[:, base:base + 128 * T].rearrange(
                        "b (p t) -> p b t", p=128),
                    in_=labi[:])
                nc.sync.dma_start(
                    out=reg_o[:, base:base + 128 * T].rearrange(
                        "b (p t) c -> p b t c", p=128),
                    in_=rg[:])

            pend = []
            rcp_next = phase_rcp(0)
            for n in range(NT):
                rcp_cur = rcp_next
                if n + 1 < NT:
                    rcp_next = phase_rcp(n + 1)
                pend.append((n,) + phase_ab(n, rcp_cur))
                if len(pend) > 2:
                    phase_c(*pend.pop(0))
            for item in pend:
                phase_c(*item)

    from concourse.library_overlay import lower_extended_insts
    lower_extended_insts(nc)
    _split_multi_waits(nc)
    _BUILD_CACHE["nc"] = nc
    return nc


def _split_multi_waits(nc):
    """The TPB 64B instruction encoding has a single semaphore-wait slot.
    Tile's sem assignment can attach several waits to one instruction; walrus
    rejects those ("Too many sync wait commands"). Hoist all but one wait onto
    wait-only NoOps on the same engine immediately before the instruction."""
    import bass_rust

    for fn in nc.m.functions:
        for blk in fn.blocks:
            out = []
            for ins in blk.instructions:
                si = ins.sync_info
                if si is not None and si.on_wait and len(si.on_wait) > 1:
                    waits = list(si.on_wait)
                    for j, w in enumerate(waits[:-1]):
                        nop = bass_rust.InstNoOp(
                            name=f"{ins.name}-w{j}", engine=ins.engine,
                            ins=[], outs=[],
                            sync_info=mybir.SyncInfo(on_wait=[w], on_update=[]),
                        )
                        out.append(nop)
                    ins.sync_info = mybir.SyncInfo(
                        on_wait=[waits[-1]], on_update=list(si.on_update or []))
                out.append(ins)
            blk.instructions[:] = out


def _prep_anchor_ext(anchors):
    a = np.zeros((A_PAD, 4), np.float32)
    a[:A_TOT] = anchors
    a[A_TOT:, 0] = 1e8
    a[A_TOT:, 1] = 1e8
    a[A_TOT:, 2] = 1e8 + 10.0
    a[A_TOT:, 3] = 1e8 + 10.0
    x1, y1, x2, y2 = a[:, 0], a[:, 1], a[:, 2], a[:, 3]
    ew = x2 - x1 + 1.0
    eh = y2 - y1 + 1.0
    ext = np.zeros((A_PAD, NCOL), np.float32)
    ext[:, 0] = x2 + 1.0
    ext[:, 1] = -x1
    ext[:, 2] = y2 + 1.0
    ext[:, 3] = -y1
    ext[:, 4] = ew * eh
    ext[:, 5] = x1 + 0.5 * ew          # ecx
    ext[:, 6] = y1 + 0.5 * eh          # ecy
    ext[:, 7] = 1.0 / ew
    ext[:, 8] = 1.0 / eh
    ext[:, 9] = np.log(ew)
    ext[:, 10] = np.log(eh)
    ext[:, 11] = 1.0
    return ext


def _prep_gt_ext(bb, ids):
    # bb [n,G,4] f32, ids [n,G] -> [n, NCOL, G] f32
    n = bb.shape[0]
    x1, y1, x2, y2 = bb[..., 0], bb[..., 1], bb[..., 2], bb[..., 3]
    gw = x2 - x1 + 1.0
    gh = y2 - y1 + 1.0
    ext = np.zeros((n, NCOL, G), np.float32)
    ext[:, 0] = x2 + 1.0
    ext[:, 1] = -x1
    ext[:, 2] = y2 + 1.0
    ext[:, 3] = -y1
    ext[:, 4] = gw * gh
    ext[:, 5] = x1 + 0.5 * gw          # gcx
    ext[:, 6] = y1 + 0.5 * gh          # gcy
    ext[:, 7] = np.log(gw)
    ext[:, 8] = np.log(gh)
    ext[:, 9] = ids.astype(np.float32)
    return ext


def _exact_row_iou(anch_rows, gt_box):
    # anch_rows [T,4], gt_box [4] — float32, reference-order arithmetic
    one = np.float32(1.0)
    ax1, ay1, ax2, ay2 = (anch_rows[:, k] for k in range(4))
    gx1, gy1, gx2, gy2 = (np.float32(gt_box[k]) for k in range(4))
    iw = np.clip(np.minimum(ax2, gx2) - np.maximum(ax1, gx1) + one, 0.0, None)
    ih = np.clip(np.minimum(ay2, gy2) - np.maximum(ay1, gy1) + one, 0.0, None)
    area_a = (ax2 - ax1 + one) * (ay2 - ay1 + one)
    area_g = (gx2 - gx1 + one) * (gy2 - gy1 + one)
    inter = iw * ih
    return inter / (area_a + area_g - inter)


def kernel(bb_coord, bird_ids, anchors_l0, anchors_l1, anchors_l2, anchors_l3,
           anchors_l4, _trace=False):
    bb_coord = np.asarray(bb_coord, np.float32)
    bird_ids_np = np.asarray(bird_ids)
    anchors = np.concatenate(
        [np.asarray(x, np.float32) for x in
         (anchors_l0, anchors_l1, anchors_l2, anchors_l3, anchors_l4)], axis=0)

    anch_ext = _prep_anchor_ext(anchors)

    nc = _build_kernel()
    in_maps = []
    for c in range(N_CORES):
        bb = bb_coord[c * NB:(c + 1) * NB]
        ids = bird_ids_np[c * NB:(c + 1) * NB]
        gte = _prep_gt_ext(bb, ids)  # [NB, NCOL, G]
        gte_rep = np.broadcast_to(gte[:, None], (NB, 128, NCOL, G)).reshape(
            NB, 128, NCOL * G).copy()
        bdg = np.zeros((128, NB * G), np.float32)
        for bimg in range(NB):
            for tw in range(4):
                # rows tw*32+g, cols bimg*20 + tw*5 + q
                bdg[tw * 32:tw * 32 + G, bimg * G + tw * 5:bimg * G + tw * 5 + 5] = \
                    gte[bimg, 5:10, :].T
        selm = np.zeros((2 * T, NB * T * G), np.float32)
        col = 0
        for bimg in range(NB):
            for t in range(T):
                selm[t, col:col + G] = 1.0
                selm[T, col:col + G] = gte[bimg, 4, :]
                col += G
        in_maps.append({"anch": anch_ext, "gte": gte_rep, "bdg": bdg,
                        "iden": np.eye(128, dtype=np.float32), "sel": selm})

    if _trace:
        sys.path.insert(0, "/root/.axon_site")
        from trn_agent_boot.trn_boot import _ntff_profile_via_ctypes
        from antenv.axon_hooks import set_axon_ntff_profile_hook
        set_axon_ntff_profile_hook(
            _ntff_profile_via_ctypes("/opt/axon/libaxon_pjrt.so"))
    res = run_bass_kernel_spmd(nc, in_maps, core_ids=list(range(N_CORES)),
                               trace=_trace)
    outs = res.results

    labels = np.zeros((B, A_TOT), np.int32)
    reg = np.zeros((B, A_TOT, 4), np.float32)
    one = np.float32(1.0)
    half = np.float32(0.5)
    for c in range(N_CORES):
        o = outs[c]
        labels[c * NB:(c + 1) * NB] = o["labels"][:, :A_TOT]
        reg[c * NB:(c + 1) * NB] = o["reg"][:, :A_TOT]
        aux = o["aux"].reshape(NT, 128, NB, G)
        for bi in range(NB):
            b = c * NB + bi
            for g in range(G):
                m = aux[:, :, bi, g]                     # [NT, 128]
                k = int(m.argmax())                      # first-index ties
                nstar, pstar = divmod(k, 128)
                base = nstar * 128 * T + pstar * T
                arow = np.empty((T, 4), np.float32)
                hi = min(base + T, A_TOT)
                arow[:hi - base] = anchors[base:hi]
                if hi - base < T:
                    arow[hi - base:] = [1e8, 1e8, 1e8 + 10.0, 1e8 + 10.0]
                ov = _exact_row_iou(arow, bb_coord[b, g])
                tstar = int(ov.argmax())
                a = base + tstar
                if a >= A_TOT:
                    continue
                ex = anchors[a]
                gtb = bb_coord[b, g]
                ew = ex[2] - ex[0] + one
                eh = ex[3] - ex[1] + one
                ecx = ex[0] + half * ew
                ecy = ex[1] + half * eh
                gw = gtb[2] - gtb[0] + one
                gh = gtb[3] - gtb[1] + one
                gcx = gtb[0] + half * gw
                gcy = gtb[1] + half * gh
                labels[b, a] = np.int32(bird_ids_np[b, g])
                reg[b, a, 0] = (gcx - ecx) / ew
                reg[b, a, 1] = (gcy - ecy) / eh
                reg[b, a, 2] = np.log(gw / ew)
                reg[b, a, 3] = np.log(gh / eh)
    if _trace:
        return (labels, reg), res
    return labels, reg
